# revision 1
# baseline (speedup 1.0000x reference)
"""CrossAttnBlock kernel for 8 Trainium2 NeuronCores.

Sharding: core c -> (batch b = c//2, token-half s = c%2), 512 query tokens
per core. Cross-attention K/V is computed fully per core (duplicated within
the pair); after cross-attention the per-core residual x2 is exchanged with
one 8-rank AllGather so each core rebuilds the partner half's self-attn K/V
locally (attention is permutation-invariant over KV tokens, so own tokens
always sit at positions 0:512).

All activations are feature-major ([feature, token]) so every linear layer
consumes natural-layout weights as the stationary matmul operand and no
on-device transposes are needed. Matmuls run in float32r (fp32 storage,
~tf32 matmul precision, 1 cycle/row at N=512). Softmax skips the max
subtraction (scores are O(1) for this problem) and gets sum-exp for free
from a ones-column appended to V. K/V are spilled to DRAM and streamed back
per head to fit SBUF.
"""
import sys

sys.path.insert(0, '/opt/trn_rl_repo')

import numpy as np
import concourse.bass as bass
from concourse import bacc
import concourse.tile as tile
from concourse import mybir

F32R = mybir.dt.float32r
F32 = mybir.dt.float32
AF = mybir.ActivationFunctionType
OP = mybir.AluOpType

N_CORES = 8
B, NSEQ, D, H, HD = 4, 1024, 1024, 16, 64
T = 512            # tokens owned per core
TF = 1024          # full token count per batch
C8 = D // 128      # feature chunks
SCALE = 1.0 / float(np.sqrt(np.float32(HD)))
EPS = 1e-6

_PROGRAM_CACHE = {}


def _rearr_w(w):
    """[Din, N] dram AP -> [128, Din//128, N] (partition, chunk, col)."""
    return w.rearrange("(c p) n -> p c n", p=128)


def _build_program():
    nc = bacc.Bacc("TRN2", target_bir_lowering=False, debug=False,
                   num_devices=N_CORES)

    dp = {}
    dp["xT"] = nc.declare_dram_parameter("xT", [D, T], F32R, isOutput=False)
    dp["kvT"] = nc.declare_dram_parameter("kvT", [D, TF], F32R, isOutput=False)
    for nm, sh in [("wq", [D, D]), ("wkv", [D, 2 * D]), ("wqkv", [D, 3 * D]),
                   ("wco", [D, D]), ("wso", [D, D]), ("w1", [D, 4 * D]),
                   ("w2", [4 * D, D])]:
        dp[nm] = nc.declare_dram_parameter(nm, sh, F32R, isOutput=False)
    for i in (1, 2, 3, 4):
        for sb in ("s", "b"):
            dp[f"ln{i}_{sb}"] = nc.declare_dram_parameter(
                f"ln{i}_{sb}", [1, D], F32, isOutput=False)
    for nm, n in [("bco", D), ("bso", D), ("b1", 4 * D), ("b2", D)]:
        dp[nm] = nc.declare_dram_parameter(nm, [1, n], F32, isOutput=False)
    dp["ones"] = nc.declare_dram_parameter("ones", [128, 128], F32R,
                                           isOutput=False)
    dp["outT"] = nc.declare_dram_parameter("outT", [D, T], F32R, isOutput=True)

    with tile.TileContext(nc) as tc:
        _emit(nc, tc, dp)
    nc.compile()
    return nc


def _emit(nc, tc, dp):
    import contextlib

    ctx = contextlib.ExitStack()
    with ctx:
        consts = ctx.enter_context(tc.tile_pool(name="consts", bufs=1))
        outer = ctx.enter_context(tc.tile_pool(name="outer", bufs=1))
        pp = ctx.enter_context(tc.tile_pool(name="pp", bufs=1, space="PSUM"))
        small = ctx.enter_context(tc.tile_pool(name="small", bufs=1))
        dramp = ctx.enter_context(tc.tile_pool(name="dramp", bufs=1,
                                               space="DRAM"))

        # ---------- constants ----------
        ones_sb = consts.tile([128, 128], F32R)
        nc.sync.dma_start(out=ones_sb[:], in_=dp["ones"][:])
        ones_col = ones_sb[:, 0:1]
        ones_row = ones_sb[0:1, :]
        eps_t = consts.tile([1, 1], F32)
        nc.vector.memset(eps_t[:], EPS)

        def load_col(name, nchunk):
            col = consts.tile([128, nchunk], F32, name=f"col_{name}")
            nc.sync.dma_start(
                out=col[:], in_=dp[name].rearrange("o (c p) -> p (o c)", p=128))
            return col

        ln_c = {f"{i}{sb}": load_col(f"ln{i}_{sb}", C8)
                for i in (1, 2, 3, 4) for sb in ("s", "b")}
        bco_c = load_col("bco", C8)
        bso_c = load_col("bso", C8)
        b1_c = load_col("b1", 32)
        b2_c = load_col("b2", C8)

        pid = nc.sync.partition_id()
        partner = (pid // 2) * 2 + (1 - pid % 2)

        # ---------- DRAM intermediates ----------
        x2_d = dramp.tile([128, C8, T], F32R, name="x2_d")          # own x2
        ag_out = dramp.tile([N_CORES, 128, C8 * T], F32R,
                            addr_space="Shared", name="ag_out")
        kT_d = dramp.tile([C8, 128, TF], F32R, name="kT_d")
        v_d = dramp.tile([128, 8, H, 65], F32R, name="v_d")
        kT2_d = dramp.tile([C8, 128, TF], F32R, name="kT2_d")
        v2_d = dramp.tile([128, 8, H, 65], F32R, name="v2_d")

        # ---------- generic helpers ----------
        def layer_norm(src_fn, dst, dst_sl, s_col, b_col, pool):
            """LN over the feature axis for 512 tokens.

            src_fn(c) -> [128, 512] fp32r AP (may DMA into a stream tile).
            dst: [128, C8, *] SBUF tile, dst_sl a 512-token slice.
            """
            stats_x = pp.tile([1, 512], F32, tag="sps", bufs=2, name="stats_x")
            stats_q = pp.tile([1, 512], F32, tag="ops", bufs=3, name="stats_q")
            for c in range(C8):
                xc = src_fn(c)
                sq = pool.tile([128, 512], F32R, tag="pt", bufs=3, name="sq")
                nc.scalar.activation(out=sq[:], in_=xc, func=AF.Square)
                nc.tensor.matmul(stats_x[:], ones_col, xc,
                                 start=(c == 0), stop=(c == C8 - 1),
                                 skip_group_check=True)
                nc.tensor.matmul(stats_q[:], ones_col, sq[:],
                                 start=(c == 0), stop=(c == C8 - 1),
                                 skip_group_check=True)
            mean = small.tile([1, 512], F32, tag="mean", bufs=1, name="mean")
            nc.vector.tensor_scalar_mul(mean[:], stats_x[:], 1.0 / D)
            var = small.tile([1, 512], F32, tag="var", bufs=1, name="var")
            nc.vector.tensor_scalar_mul(var[:], stats_q[:], 1.0 / D)
            m2 = small.tile([1, 512], F32, tag="m2", bufs=1, name="m2")
            nc.vector.tensor_mul(m2[:], mean[:], mean[:])
            nc.vector.tensor_sub(var[:], var[:], m2[:])
            std = small.tile([1, 512], F32, tag="std", bufs=1, name="std")
            nc.scalar.activation(out=std[:], in_=var[:], func=AF.Sqrt,
                                 bias=eps_t[:])
            inv = small.tile([1, 512], F32R, tag="inv", bufs=2, name="inv")
            with nc.allow_low_precision(reason="float32r is 32-bit"):
                nc.vector.reciprocal(inv[:], std[:])
            negminv = small.tile([1, 512], F32R, tag="negminv", bufs=2,
                                 name="negminv")
            nc.vector.tensor_mul(negminv[:], mean[:], inv[:])
            nc.vector.tensor_scalar_mul(negminv[:], negminv[:], -1.0)
            a0 = pp.tile([128, 512], F32, tag="mm", bufs=3, name="a0")
            nc.tensor.matmul(a0[:], ones_row, inv[:], start=True, stop=True)
            c0 = pp.tile([128, 512], F32, tag="mm", bufs=3, name="c0")
            nc.tensor.matmul(c0[:], ones_row, negminv[:], start=True, stop=True)
            for c in range(C8):
                xc = src_fn(c)
                nc.vector.tensor_mul(dst[:, c, dst_sl], xc, a0[:])
                nc.vector.tensor_add(dst[:, c, dst_sl], dst[:, c, dst_sl], c0[:])
                nc.vector.tensor_scalar(
                    dst[:, c, dst_sl], dst[:, c, dst_sl],
                    scalar1=s_col[:, c:c + 1], scalar2=b_col[:, c:c + 1],
                    op0=OP.mult, op1=OP.add)

        def dram_src(pool, dram_ap_fn, tag="lnsrc"):
            def src_fn(c):
                t = pool.tile([128, 512], F32R, tag=tag, bufs=2, name=tag)
                nc.sync.dma_start(out=t[:], in_=dram_ap_fn(c))
                return t[:]
            return src_fn

        def gemm_feat(w_dram, col_off, n_tiles, rhs_list, pool, evict, wtag="wst"):
            """Feature-major GEMM; rhs_list: [(rhs_fn(c) -> [128,512] AP, key)].
            evict(nt, key, psum_tile)."""
            wr = _rearr_w(w_dram)
            for nt in range(n_tiles):
                wt = pool.tile([128, C8, 128], F32R, tag=wtag, bufs=3,
                               name=f"w_{wtag}")
                nc.sync.dma_start(
                    out=wt[:],
                    in_=wr[:, :, col_off + nt * 128:col_off + (nt + 1) * 128])
                for (rhs_fn, key) in rhs_list:
                    ps = pp.tile([128, 512], F32, tag="mm", bufs=3, name="gps")
                    for c in range(C8):
                        nc.tensor.matmul(ps[:], wt[:, c, :], rhs_fn(c),
                                         start=(c == 0), stop=(c == C8 - 1))
                    evict(nt, key, ps)

        def spill(pool, ps_ap, dram_ap, tag="spill"):
            t = pool.tile([128, 512], F32R, tag=tag, bufs=2, name=tag)
            nc.vector.tensor_copy(out=t[:], in_=ps_ap)
            nc.sync.dma_start(out=dram_ap, in_=t[:])

        def build_v_group(pool, wv_dram_col0, src, jts, v_dram, jt_off):
            """Token-major V for a group of 128-token j-tiles, spilled to DRAM.

            wv_dram_col0: column offset of the V block inside its weight.
            src: [128, C8, 512] SBUF tile; jts: j-tile indices within src."""
            for dvh in range(2):
                wvh = pool.tile([128, C8, 512], F32R, tag="wvh", bufs=1,
                                name="wvh")
                nc.sync.dma_start(
                    out=wvh[:],
                    in_=wv_dram_col0[:, :, dvh * 512:(dvh + 1) * 512])
                for jt in jts:
                    ps = pp.tile([128, 512], F32, tag="mm", bufs=3, name="vps")
                    sl = slice((jt - jt_off) * 128, (jt - jt_off + 1) * 128)
                    for c in range(C8):
                        nc.tensor.matmul(ps[:], src[:, c, sl], wvh[:, c, :],
                                         start=(c == 0), stop=(c == C8 - 1))
                    t = pool.tile([128, 8, 64], F32R, tag="vspill", bufs=2,
                                  name="vspill")
                    nc.vector.tensor_copy(
                        out=t[:], in_=ps[:].rearrange("p (h e) -> p h e", h=8))
                    nc.sync.dma_start(
                        out=v_dram[:, jt, dvh * 8:(dvh + 1) * 8, 0:64], in_=t[:])
            for jt in jts:
                nc.sync.dma_start(out=v_dram[:, jt, :, 64], in_=ones_sb[:, 0:16])

        def attention(pool, qT, kT_dram, v_dram, oT):
            for h in range(H):
                ch, off = h // 2, (h % 2) * 64
                if off == 0:
                    kth = pool.tile([128, TF], F32R, tag="kth", bufs=2,
                                    name="kth")
                    nc.sync.dma_start(out=kth[:], in_=kT_dram[ch, :, :])
                vh = pool.tile([128, 8, 65], F32R, tag="vh", bufs=2, name="vh")
                nc.sync.dma_start(out=vh[:], in_=v_dram[:, :, h, :])
                o_ps = pp.tile([65, 512], F32, tag="ops", bufs=3, name="ops")
                for jt in range(8):
                    s_ps = pp.tile([128, 512], F32, tag="sps", bufs=2, name="sps")
                    nc.tensor.matmul(s_ps[:],
                                     kth[off:off + 64, jt * 128:(jt + 1) * 128],
                                     qT[off:off + 64, ch, :],
                                     start=True, stop=True)
                    pt = pool.tile([128, 512], F32R, tag="pt", bufs=3, name="pt")
                    nc.scalar.activation(out=pt[:], in_=s_ps[:], func=AF.Exp,
                                         scale=SCALE)
                    nc.tensor.matmul(o_ps[:], vh[:, jt, :], pt[:],
                                     start=(jt == 0), stop=(jt == 7),
                                     skip_group_check=True)
                zrec = small.tile([1, 512], F32, tag="zrec", bufs=2, name="zrec")
                nc.vector.reciprocal(zrec[:], o_ps[64:65, :])
                zd = dramp.tile([1, 512], F32, tag="zd", bufs=2, name="zd")
                nc.sync.dma_start(out=zd[:], in_=zrec[:])
                zb = pool.tile([64, 512], F32, tag="zb", bufs=2, name="zb")
                zsrc = bass.AP(tensor=zd.tensor, offset=zd.offset,
                               ap=[[0, 64]] + list(zd.ap[1:]))
                nc.sync.dma_start(out=zb[:], in_=zsrc)
                nc.vector.tensor_mul(oT[off:off + 64, ch, :], o_ps[0:64, :],
                                     zb[:])

        # ---------- load x ----------
        x1 = outer.tile([128, C8, T], F32R, tag="res512", bufs=2, name="x1")
        nc.sync.dma_start(out=x1[:],
                          in_=dp["xT"].rearrange("(c p) t -> p c t", p=128))

        # ================= Phase 1: cross-attention =================
        with tc.tile_pool(name="p1", bufs=1) as p1:
            q_in = p1.tile([128, C8, T], F32R, name="q_in")
            layer_norm(lambda c: x1[:, c, :], q_in, slice(0, 512),
                       ln_c["1s"], ln_c["1b"], p1)

            qT1 = p1.tile([128, C8, T], F32R, name="qT1")

            def ev_qT(nt, key, ps):
                nc.vector.tensor_copy(out=qT1[:, nt, :], in_=ps[:])

            gemm_feat(dp["wq"], 0, C8, [(lambda c: q_in[:, c, :], 0)], p1, ev_qT)

            kvT_r = dp["kvT"].rearrange("(c p) t -> p c t", p=128)
            kv_in = p1.tile([128, C8, TF], F32R, name="kv_in")

            def ev_kT(nt, th, ps):
                spill(p1, ps[:], kT_d[nt, :, th * 512:(th + 1) * 512])

            wv1 = _rearr_w(dp["wkv"])[:, :, D:2 * D]
            for th in range(2):
                sl = slice(th * 512, (th + 1) * 512)
                layer_norm(dram_src(p1, lambda c, sl=sl: kvT_r[:, c, sl]),
                           kv_in, sl, ln_c["2s"], ln_c["2b"], p1)
                gemm_feat(dp["wkv"], 0, C8,
                          [(lambda c, sl=sl: kv_in[:, c, sl], th)], p1, ev_kT)
                build_v_group(p1, wv1, kv_in[:, :, sl], range(th * 4, th * 4 + 4),
                              v_d, th * 4)

            oT1 = p1.tile([128, C8, T], F32R, name="oT1")
            attention(p1, qT1, kT_d, v_d, oT1)

            # x2 = x1 + Wco @ o + bco  (straight to DRAM; it is the AG input)
            def ev_x2(nt, key, ps):
                t = p1.tile([128, 512], F32R, tag="spill", bufs=2, name="x2s")
                nc.vector.scalar_tensor_tensor(
                    out=t[:], in0=ps[:], scalar=bco_c[:, nt:nt + 1],
                    in1=x1[:, nt, :], op0=OP.add, op1=OP.add)
                nc.sync.dma_start(out=x2_d[:, nt, :], in_=t[:])

            gemm_feat(dp["wco"], 0, C8, [(lambda c: oT1[:, c, :], 0)], p1, ev_x2)

        # ================= x2 exchange (8-rank AllGather) =================
        nc.gpsimd.collective_compute(
            "AllGather", OP.bypass,
            ins=[x2_d[:].rearrange("p c t -> p (c t)")],
            outs=[ag_out[:]],
            replica_groups=[list(range(N_CORES))])
        ag_rem = ag_out[bass.ds(partner, 1), :, :].rearrange(
            "o p (c t) -> p (o c) t", c=C8)

        # ================= Phase 2: self-attention =================
        with tc.tile_pool(name="p2", bufs=1) as p2:
            s_own = p2.tile([128, C8, T], F32R, name="s_own")
            layer_norm(dram_src(p2, lambda c: x2_d[:, c, :]), s_own,
                       slice(0, 512), ln_c["3s"], ln_c["3b"], p2)

            qT2 = p2.tile([128, C8, T], F32R, name="qT2")

            def ev_qT2(nt, key, ps):
                nc.vector.tensor_copy(out=qT2[:, nt, :], in_=ps[:])

            gemm_feat(dp["wqkv"], 0, C8, [(lambda c: s_own[:, c, :], 0)], p2,
                      ev_qT2)

            def ev_kT2(nt, half, ps):
                spill(p2, ps[:], kT2_d[nt, :, half * 512:(half + 1) * 512])

            gemm_feat(dp["wqkv"], D, C8, [(lambda c: s_own[:, c, :], 0)], p2,
                      ev_kT2)

            wv2 = _rearr_w(dp["wqkv"])[:, :, 2 * D:3 * D]
            build_v_group(p2, wv2, s_own, range(0, 4), v2_d, 0)

            # remote half (depends on the AllGather)
            s_rem = p2.tile([128, C8, T], F32R, name="s_rem")
            layer_norm(dram_src(p2, lambda c: ag_rem[:, c, :], tag="lnsrc2"),
                       s_rem, slice(0, 512), ln_c["3s"], ln_c["3b"], p2)
            gemm_feat(dp["wqkv"], D, C8, [(lambda c: s_rem[:, c, :], 1)], p2,
                      ev_kT2)
            build_v_group(p2, wv2, s_rem, range(4, 8), v2_d, 4)

            oT2 = p2.tile([128, C8, T], F32R, name="oT2")
            attention(p2, qT2, kT2_d, v2_d, oT2)

            x3 = outer.tile([128, C8, T], F32R, tag="res512", bufs=2, name="x3")

            def ev_x3(nt, key, ps):
                x2c = p2.tile([128, 512], F32R, tag="lnsrc", bufs=2, name="x2c")
                nc.sync.dma_start(out=x2c[:], in_=x2_d[:, nt, :])
                nc.vector.scalar_tensor_tensor(
                    out=x3[:, nt, :], in0=ps[:], scalar=bso_c[:, nt:nt + 1],
                    in1=x2c[:], op0=OP.add, op1=OP.add)

            gemm_feat(dp["wso"], 0, C8, [(lambda c: oT2[:, c, :], 0)], p2, ev_x3)

        # ================= Phase 3: MLP =================
        with tc.tile_pool(name="p3", bufs=1) as p3:
            m_in = p3.tile([128, C8, T], F32R, name="m_in")
            layer_norm(lambda c: x3[:, c, :], m_in, slice(0, 512),
                       ln_c["4s"], ln_c["4b"], p3)

            hT = p3.tile([128, 32, T], F32R, name="hT")

            def ev_h(ht, key, ps):
                nc.scalar.activation(out=hT[:, ht, :], in_=ps[:],
                                     func=AF.Gelu_apprx_tanh,
                                     bias=b1_c[:, ht:ht + 1], scale=1.0)

            gemm_feat(dp["w1"], 0, 32, [(lambda c: m_in[:, c, :], 0)], p3, ev_h)

            w2r = _rearr_w(dp["w2"])  # [128, 32, D]
            outT_r = dp["outT"].rearrange("(c p) t -> p c t", p=128)
            for nt in range(C8):
                w2t = p3.tile([128, 32, 128], F32R, tag="w2t", bufs=2,
                              name="w2t")
                nc.sync.dma_start(out=w2t[:],
                                  in_=w2r[:, :, nt * 128:(nt + 1) * 128])
                ps = pp.tile([128, 512], F32, tag="mm", bufs=3, name="ops2")
                for kk in range(32):
                    nc.tensor.matmul(ps[:], w2t[:, kk, :], hT[:, kk, :],
                                     start=(kk == 0), stop=(kk == 31))
                ot = p3.tile([128, 512], F32R, tag="spill", bufs=2, name="ot")
                nc.vector.tensor_scalar_add(ot[:], ps[:],
                                            scalar1=b2_c[:, nt:nt + 1])
                nc.sync.dma_start(out=outT_r[:, nt, :], in_=ot[:])


def _get_program():
    if "nc" not in _PROGRAM_CACHE:
        _PROGRAM_CACHE["nc"] = _build_program()
    return _PROGRAM_CACHE["nc"]


def kernel(**inputs) -> np.ndarray:
    from concourse.bass_utils import run_bass_kernel_spmd

    nc = _get_program()

    x = np.asarray(inputs["x"], np.float32)
    key_val = np.asarray(inputs["key_val"], np.float32)
    f32 = lambda a: np.ascontiguousarray(np.asarray(a, np.float32))
    shared = {
        "wq": f32(inputs["Wq"]), "wkv": f32(inputs["Wkv"]),
        "wqkv": f32(inputs["Wqkv"]), "wco": f32(inputs["Wco"]),
        "wso": f32(inputs["Wso"]), "w1": f32(inputs["W1"]),
        "w2": f32(inputs["W2"]),
        "ln1_s": f32(inputs["ln1_s"])[None, :], "ln1_b": f32(inputs["ln1_b"])[None, :],
        "ln2_s": f32(inputs["ln2_s"])[None, :], "ln2_b": f32(inputs["ln2_b"])[None, :],
        "ln3_s": f32(inputs["ln3_s"])[None, :], "ln3_b": f32(inputs["ln3_b"])[None, :],
        "ln4_s": f32(inputs["ln4_s"])[None, :], "ln4_b": f32(inputs["ln4_b"])[None, :],
        "bco": f32(inputs["bco"])[None, :], "bso": f32(inputs["bso"])[None, :],
        "b1": f32(inputs["b1"])[None, :], "b2": f32(inputs["b2"])[None, :],
        "ones": np.ones((128, 128), np.float32),
    }
    in_maps = []
    for c in range(N_CORES):
        b, s = c // 2, c % 2
        m = dict(shared)
        m["xT"] = np.ascontiguousarray(x[b, s * T:(s + 1) * T, :].T)
        m["kvT"] = np.ascontiguousarray(key_val[b].T)
        in_maps.append(m)

    res = run_bass_kernel_spmd(nc, in_maps, list(range(N_CORES)))
    _PROGRAM_CACHE["last_result"] = res

    out = np.empty((B, NSEQ, D), np.float32)
    for c in range(N_CORES):
        b, s = c // 2, c % 2
        out[b, s * T:(s + 1) * T, :] = res.results[c]["outT"].T
    return out



# revision 15
# speedup vs baseline: 1.7808x; 1.7808x over previous
"""CrossAttnBlock kernel for 8 Trainium2 NeuronCores.

Sharding: core c -> (batch b = c//2, token-half s = c%2), 512 query tokens
per core. Cross-attention K/V is computed fully per core (duplicated within
the pair); after cross-attention the per-core residual x2 is exchanged with
a PAIR-wise AllGather (bf16) so each core rebuilds the partner half's
self-attn K/V locally.

All activations are feature-major ([feature, token]). Weights are pre-tiled
on the host into contiguous [nt, 128, C8*128] bf16 blocks so each weight
DMA is a single large contiguous transfer. LayerNorm affine params are
folded into the weights on the host (diag(s) @ W row-scaling); the LN bias
contributions become per-output-feature biases (K-projection bias drops
exactly — softmax is invariant to per-query score shifts; V-projection bias
passes through the normalized softmax unchanged and folds into the output
projection bias). K/V/x2 stay SBUF-resident. Softmax uses a ones-column in
V for the sum-exp, reciprocal_approx_fast on DVE, and a PE-matmul broadcast
of 1/z (no DRAM round trips).
"""
import sys

sys.path.insert(0, '/opt/trn_rl_repo')

import ml_dtypes
import numpy as np
import concourse.bass as bass
from concourse import bacc
import concourse.tile as tile
from concourse import mybir

F32R = mybir.dt.float32r
F32 = mybir.dt.float32
BF16 = mybir.dt.bfloat16
AF = mybir.ActivationFunctionType
OP = mybir.AluOpType

N_CORES = 8
B, NSEQ, D, H, HD = 4, 1024, 1024, 16, 64
T = 512            # tokens owned per core
TF = 1024          # full token count per batch
C8 = D // 128      # feature chunks
SCALE = 1.0 / float(np.sqrt(np.float32(HD)))
EPS = 1e-6

_PROGRAM_CACHE = {}
import os
KDBG = os.environ.get("KDBG", "")


def _build_program():
    nc = bacc.Bacc("TRN2", target_bir_lowering=False, debug=False,
                   num_devices=N_CORES)

    dp = {}
    dp["xT"] = nc.declare_dram_parameter("xT", [128, C8, T], F32R,
                                         isOutput=False)
    dp["kvT"] = nc.declare_dram_parameter("kvT", [128, C8, TF], BF16,
                                          isOutput=False)
    # pre-tiled bf16 weights: [nt, 128, C8(contraction chunks), 128]
    for nm, ntiles, nchunk in [("wq", 8, 8), ("wk1", 8, 8), ("wco", 8, 8),
                               ("wq2", 8, 8), ("wk2", 8, 8), ("wso", 8, 8),
                               ("w1", 32, 8), ("w2", 8, 32)]:
        dp[nm] = nc.declare_dram_parameter(nm, [ntiles, 128, nchunk, 128],
                                           BF16, isOutput=False)
    # V-projection weights in moving layout [128, C8, 1024]
    dp["wv1"] = nc.declare_dram_parameter("wv1", [128, C8, TF], BF16,
                                          isOutput=False)
    dp["wv2"] = nc.declare_dram_parameter("wv2", [128, C8, TF], BF16,
                                          isOutput=False)
    for nm, n in [("bq1", D), ("bq2", D), ("bco", D), ("bso", D),
                  ("b1", 4 * D), ("b2", D)]:
        dp[nm] = nc.declare_dram_parameter(nm, [1, n], F32, isOutput=False)
    dp["ones"] = nc.declare_dram_parameter("ones", [128, 128], F32R,
                                           isOutput=False)
    dp["outT"] = nc.declare_dram_parameter("outT", [128, C8, T], F32R,
                                           isOutput=True)

    with tile.TileContext(nc) as tc:
        _emit(nc, tc, dp)
    nc.compile()
    return nc


def _emit(nc, tc, dp):
    import contextlib

    ctx = contextlib.ExitStack()
    with ctx:
        consts = ctx.enter_context(tc.tile_pool(name="consts", bufs=1))
        outer = ctx.enter_context(tc.tile_pool(name="outer", bufs=1))
        work = ctx.enter_context(tc.tile_pool(name="work", bufs=1))
        pp = ctx.enter_context(tc.tile_pool(name="pp", bufs=1, space="PSUM"))
        small = ctx.enter_context(tc.tile_pool(name="small", bufs=1))
        dramp = ctx.enter_context(tc.tile_pool(name="dramp", bufs=1,
                                               space="DRAM"))

        # ---------- constants ----------
        ones_sb = consts.tile([128, 128], F32R)
        nc.sync.dma_start(out=ones_sb[:], in_=dp["ones"][:])
        ones_col = ones_sb[:, 0:1]
        ones_row = ones_sb[0:1, :]
        ones_bf = consts.tile([128, 128], BF16)
        nc.vector.memset(ones_bf[:], 1.0)
        eps_t = consts.tile([1, 1], F32)
        nc.vector.memset(eps_t[:], EPS)

        def load_col(name, nchunk):
            col = consts.tile([128, nchunk], F32, name=f"col_{name}")
            nc.sync.dma_start(
                out=col[:], in_=dp[name].rearrange("o (c p) -> p (o c)", p=128))
            return col

        bq1_c = load_col("bq1", C8)
        bq2_c = load_col("bq2", C8)
        bco_c = load_col("bco", C8)
        bso_c = load_col("bso", C8)
        b1_c = load_col("b1", 32)
        b2_c = load_col("b2", C8)

        pid = nc.sync.partition_id()
        partner_slot = 1 - pid % 2

        # ---------- DRAM intermediates ----------
        x2bf_d = dramp.tile([128, C8 * T], BF16, name="x2bf_d")
        ag2 = dramp.tile([2, 128, C8 * T], BF16, name="ag2")

        # ---------- generic helpers ----------
        def layer_norm(src_fn, dst, dst_sl, pool, src_bf=False):
            """Pure LN (no affine) over the feature axis for 512 tokens.

            src_fn(c) -> [128, 512] AP. dst: [128, C8, *] SBUF tile.
            src_bf: True when the source tiles are bf16 (matmul operands
            must match width, so the stats lhsT must be bf16 too).
            """
            stat1 = ones_bf[:, 0:1] if src_bf else ones_col
            stats_x = pp.tile([1, 512], F32, tag="sps", bufs=2, name="stats_x")
            stats_q = pp.tile([1, 512], F32, tag="ops", bufs=2, name="stats_q")
            srcs = []
            for c in range(C8):
                xc = src_fn(c)
                srcs.append(xc)
                sq = work.tile([128, 512], F32R, tag="sq", bufs=3, name="sq")
                nc.scalar.activation(out=sq[:], in_=xc, func=AF.Square)
                nc.tensor.matmul(stats_x[:], stat1, xc,
                                 start=(c == 0), stop=(c == C8 - 1),
                                 skip_group_check=True)
                nc.tensor.matmul(stats_q[:], ones_col, sq[:],
                                 start=(c == 0), stop=(c == C8 - 1),
                                 skip_group_check=True)
            mean = small.tile([1, 512], F32, tag="mean", bufs=1, name="mean")
            nc.vector.tensor_scalar_mul(mean[:], stats_x[:], 1.0 / D)
            var = small.tile([1, 512], F32, tag="var", bufs=1, name="var")
            nc.vector.tensor_scalar_mul(var[:], stats_q[:], 1.0 / D)
            m2 = small.tile([1, 512], F32, tag="m2", bufs=1, name="m2")
            nc.vector.tensor_mul(m2[:], mean[:], mean[:])
            nc.vector.tensor_sub(var[:], var[:], m2[:])
            std = small.tile([1, 512], F32, tag="std", bufs=1, name="std")
            nc.scalar.activation(out=std[:], in_=var[:], func=AF.Sqrt,
                                 bias=eps_t[:])
            inv = small.tile([1, 512], F32, tag="inv", bufs=2, name="inv")
            nc.vector.reciprocal_approx_fast(out=inv[:], in_=std[:])
            inv_bf = small.tile([1, 512], BF16, tag="inv_bf", bufs=2,
                                name="inv_bf")
            nc.scalar.copy(out=inv_bf[:], in_=inv[:])
            negminv = small.tile([1, 512], BF16, tag="negminv", bufs=2,
                                 name="negminv")
            nc.vector.scalar_tensor_tensor(
                out=negminv[:], in0=mean[:], scalar=-1.0, in1=inv[:],
                op0=OP.mult, op1=OP.mult)
            a0 = pp.tile([128, 512], F32, tag="mm", bufs=3, name="a0")
            nc.tensor.matmul(a0[:], ones_bf[0:1, :], inv_bf[:],
                             start=True, stop=True)
            c0 = pp.tile([128, 512], F32, tag="mm", bufs=3, name="c0")
            nc.tensor.matmul(c0[:], ones_bf[0:1, :], negminv[:],
                             start=True, stop=True)
            for c in range(C8):
                nc.vector.tensor_mul(dst[:, c, dst_sl], srcs[c], a0[:])
                nc.vector.tensor_add(dst[:, c, dst_sl], dst[:, c, dst_sl],
                                     c0[:])

        def gemm_feat(w_dram, n_tiles, rhs_list, evict):
            """Feature-major GEMM. w_dram: [nt, 128, C8, 128] bf16 tiles.
            rhs_list: [(rhs_fn(c) -> [128,512] AP, key)]. evict(nt, key, ps)."""
            for nt in range(n_tiles):
                wt = work.tile([128, C8, 128], BF16, tag="wt", bufs=3,
                               name="wt")
                nc.sync.dma_start(out=wt[:], in_=w_dram[nt])
                for (rhs_fn, key) in rhs_list:
                    ps = pp.tile([128, 512], F32, tag="mm", bufs=3, name="gps")
                    for c in range(C8):
                        nc.tensor.matmul(ps[:], wt[:, c, :], rhs_fn(c),
                                         start=(c == 0), stop=(c == C8 - 1))
                    evict(nt, key, ps)

        def build_v(src, jts, wv_sb, v_sb):
            """Token-major V into SBUF. src: [128, C8, 512] bf16 (LN out);
            jts: j-tile indices (token blocks of 128) relative to src."""
            for jt in jts:
                sl = slice((jt % 4) * 128, (jt % 4 + 1) * 128)
                ps0 = pp.tile([128, 512], F32, tag="mm", bufs=3, name="vps0")
                ps1 = pp.tile([128, 512], F32, tag="mm", bufs=3, name="vps1")
                for c in range(C8):
                    lhsT = src[:, c, sl]
                    nc.tensor.matmul(ps0[:], lhsT, wv_sb[:, c, 0:512],
                                     start=(c == 0), stop=(c == C8 - 1),
                                     skip_group_check=True)
                    nc.tensor.matmul(ps1[:], lhsT, wv_sb[:, c, 512:1024],
                                     start=(c == 0), stop=(c == C8 - 1),
                                     skip_group_check=True)
                nc.vector.tensor_copy(
                    out=v_sb[:, jt, 0:8, 0:64],
                    in_=ps0[:].rearrange("p (h e) -> p h e", h=8))
                nc.vector.tensor_copy(
                    out=v_sb[:, jt, 8:16, 0:64],
                    in_=ps1[:].rearrange("p (h e) -> p h e", h=8))

        def attention(qT, kT_sb, v_sb, oT, aname=""):
            for h in range(H):
                ch, off = h // 2, (h % 2) * 64
                o_ps = pp.tile([65, 512], F32, tag="ops", bufs=2, name="o_ps")
                for jt in range(8):
                    s_ps = pp.tile([128, 512], F32, tag="sps", bufs=2,
                                   name="s_ps")
                    nc.tensor.matmul(
                        s_ps[:],
                        kT_sb[off:off + 64, ch, jt * 128:(jt + 1) * 128],
                        qT[off:off + 64, ch, :], start=True, stop=True)
                    pt = work.tile([128, 512], BF16, tag="pt", bufs=3,
                                   name="pt")
                    nc.scalar.activation(out=pt[:], in_=s_ps[:], func=AF.Exp,
                                         scale=SCALE)
                    nc.tensor.matmul(o_ps[:], v_sb[:, jt, h, 0:65], pt[:],
                                     start=(jt == 0), stop=(jt == 7),
                                     skip_group_check=True)
                if KDBG == aname + "oz0" and h == 0:
                    st = work.tile([128, 512], F32R, tag="dbgst", bufs=2,
                                   name="dbgst")
                    nc.scalar.copy(out=st[0:65, :], in_=o_ps[:])
                    nc.sync.dma_start(out=dp["outT"][0:65, 0, :],
                                      in_=st[0:65, :])
                zrow = small.tile([1, 512], F32, tag="zrow", bufs=2,
                                  name="zrow")
                nc.scalar.copy(out=zrow[:], in_=o_ps[64:65, :])
                zrec = small.tile([1, 512], F32, tag="zrec", bufs=2,
                                  name="zrec")
                nc.vector.reciprocal_approx_fast(out=zrec[:], in_=zrow[:])
                if KDBG == aname + "zr0" and h == 0:
                    st = work.tile([128, 512], F32R, tag="dbgst", bufs=2,
                                   name="dbgst")
                    nc.scalar.copy(out=st[0:1, :], in_=zrec[:])
                    nc.sync.dma_start(out=dp["outT"][0:1, 0, :],
                                      in_=st[0:1, :])
                zrec_bf = small.tile([1, 512], BF16, tag="zrec_bf", bufs=2,
                                     name="zrec_bf")
                nc.scalar.copy(out=zrec_bf[:], in_=zrec[:])
                zb = pp.tile([64, 512], F32, tag="mm", bufs=3, name="zb")
                nc.tensor.matmul(zb[:], ones_bf[0:1, 0:64], zrec_bf[:],
                                 start=True, stop=True)
                nc.scalar.copy(out=oT[off:off + 64, ch, :],
                               in_=o_ps[0:64, :])
                nc.vector.tensor_mul(oT[off:off + 64, ch, :],
                                     oT[off:off + 64, ch, :], zb[:])

        def dbg_dump(name, ap_fn, n=C8, bf=False):
            """If KDBG==name, copy chunks into outT."""
            if KDBG != name:
                return False
            for c in range(n):
                st = work.tile([128, 512], F32R, tag="dbgst", bufs=2,
                               name="dbgst")
                nc.scalar.copy(out=st[:], in_=ap_fn(c))
                nc.sync.dma_start(out=dp["outT"][:, c % C8, :], in_=st[:])
            return True

        # ---------- load x ----------
        x1 = outer.tile([128, C8, T], F32R, tag="res", bufs=2, name="x1")
        nc.sync.dma_start(out=x1[:], in_=dp["xT"][:])

        # ================= Phase 1: cross-attention =================
        with tc.tile_pool(name="p1", bufs=1) as p1:
            q_in = p1.tile([128, C8, T], BF16, name="q_in")
            layer_norm(lambda c: x1[:, c, :], q_in, slice(0, 512), p1)

            kv_in = p1.tile([128, C8, TF], BF16, name="kv_in")

            kvh0 = p1.tile([128, C8, 512], BF16, tag="kvh", bufs=1,
                           name="kvh")
            nc.sync.dma_start(out=kvh0[:], in_=dp["kvT"][:, :, 0:512])
            layer_norm(lambda c: kvh0[:, c, :], kv_in, slice(0, 512), p1, src_bf=True)

            qT1 = p1.tile([128, C8, T], BF16, name="qT1")

            def ev_qT(col):
                def ev(nt, key, ps):
                    nc.scalar.activation(out=qT1[:, nt, :], in_=ps[:],
                                         func=AF.Identity,
                                         bias=col[:, nt:nt + 1])
                return ev

            gemm_feat(dp["wq"], C8, [(lambda c: q_in[:, c, :], 0)],
                      ev_qT(bq1_c))

            kvh1 = p1.tile([128, C8, 512], BF16, tag="kvh", bufs=1,
                           name="kvh")
            nc.sync.dma_start(out=kvh1[:], in_=dp["kvT"][:, :, 512:1024])
            layer_norm(lambda c: kvh1[:, c, :], kv_in, slice(512, 1024), p1, src_bf=True)

            kT_sb = p1.tile([128, C8, TF], BF16, name="kT_sb")

            def ev_kT(nt, th, ps):
                nc.scalar.activation(
                    out=kT_sb[:, nt, th * 512:(th + 1) * 512], in_=ps[:],
                    func=AF.Copy)

            gemm_feat(dp["wk1"], C8,
                      [(lambda c: kv_in[:, c, 0:512], 0),
                       (lambda c: kv_in[:, c, 512:1024], 1)], ev_kT)

            v_sb = p1.tile([128, 8, H, 65], BF16, name="v_sb")
            nc.vector.memset(v_sb[:, :, :, 64], 1.0)
            wv_sb = work.tile([128, C8, TF], BF16, tag="wv", bufs=1,
                              name="wv_sb")
            nc.sync.dma_start(out=wv_sb[:], in_=dp["wv1"][:])
            build_v(kv_in[:, :, 0:512], range(0, 4), wv_sb, v_sb)
            build_v(kv_in[:, :, 512:1024], range(4, 8), wv_sb, v_sb)

            oT1 = p1.tile([128, C8, T], BF16, name="oT1")
            attention(qT1, kT_sb, v_sb, oT1, aname="a1")

            # x2 = x1 + Wco @ o + bco'
            x2 = outer.tile([128, C8, T], F32R, tag="res", bufs=2, name="x2")

            def ev_x2(nt, key, ps):
                nc.vector.scalar_tensor_tensor(
                    out=x2[:, nt, :], in0=ps[:], scalar=bco_c[:, nt:nt + 1],
                    in1=x1[:, nt, :], op0=OP.add, op1=OP.add)
                x2bf = work.tile([128, 512], BF16, tag="x2bf", bufs=2,
                                 name="x2bf")
                nc.scalar.activation(out=x2bf[:], in_=x2[:, nt, :],
                                     func=AF.Copy)
                nc.sync.dma_start(
                    out=x2bf_d[:, nt * 512:(nt + 1) * 512], in_=x2bf[:])

            gemm_feat(dp["wco"], C8, [(lambda c: oT1[:, c, :], 0)], ev_x2)
            dbg_dump("q_in", lambda c: q_in[:, c, :])
            dbg_dump("kv_in", lambda c: kv_in[:, c, 0:512])
            dbg_dump("kv_in1", lambda c: kv_in[:, c, 512:1024])
            dbg_dump("qT1", lambda c: qT1[:, c, :])
            dbg_dump("kT0", lambda c: kT_sb[:, c, 0:512])
            dbg_dump("oT1", lambda c: oT1[:, c, :])
            dbg_dump("x2", lambda c: x2[:, c, :])

        # ================= x2 exchange (pair AllGather, bf16) ============
        nc.gpsimd.collective_compute(
            "AllGather", OP.bypass,
            ins=[x2bf_d[:]],
            outs=[ag2[:]],
            replica_groups=[[0, 1], [2, 3], [4, 5], [6, 7]])

        # ================= Phase 2: self-attention =================
        with tc.tile_pool(name="p2", bufs=1) as p2:
            s_own = p2.tile([128, C8, T], BF16, name="s_own")
            layer_norm(lambda c: x2[:, c, :], s_own, slice(0, 512), p2)

            qT2 = p2.tile([128, C8, T], BF16, name="qT2")

            def ev_qT2(nt, key, ps):
                nc.scalar.activation(out=qT2[:, nt, :], in_=ps[:],
                                     func=AF.Identity,
                                     bias=bq2_c[:, nt:nt + 1])

            gemm_feat(dp["wq2"], C8, [(lambda c: s_own[:, c, :], 0)], ev_qT2)

            kT2_sb = p2.tile([128, C8, TF], BF16, name="kT2_sb")

            def ev_kT2(nt, half, ps):
                nc.scalar.activation(
                    out=kT2_sb[:, nt, half * 512:(half + 1) * 512], in_=ps[:],
                    func=AF.Copy)

            gemm_feat(dp["wk2"], C8, [(lambda c: s_own[:, c, :], 0)], ev_kT2)

            v2_sb = p2.tile([128, 8, H, 65], BF16, name="v2_sb")
            nc.vector.memset(v2_sb[:, :, :, 64], 1.0)
            wv2_sb = work.tile([128, C8, TF], BF16, tag="wv", bufs=1,
                               name="wv2_sb")
            nc.sync.dma_start(out=wv2_sb[:], in_=dp["wv2"][:])
            build_v(s_own, range(0, 4), wv2_sb, v2_sb)

            # remote half (depends on the AllGather)
            ag_sb = p2.tile([128, C8, T], BF16, name="ag_sb")
            nc.sync.dma_start(
                out=ag_sb[:],
                in_=ag2[bass.ds(partner_slot, 1), :, :].rearrange(
                    "o p (c t) -> p (o c) t", c=C8))
            s_rem = p2.tile([128, C8, T], BF16, name="s_rem")
            layer_norm(lambda c: ag_sb[:, c, :], s_rem, slice(0, 512), p2, src_bf=True)
            gemm_feat(dp["wk2"], C8, [(lambda c: s_rem[:, c, :], 1)], ev_kT2)
            build_v(s_rem, range(4, 8), wv2_sb, v2_sb)

            oT2 = p2.tile([128, C8, T], BF16, name="oT2")
            attention(qT2, kT2_sb, v2_sb, oT2, aname="a2")

            x3 = outer.tile([128, C8, T], F32R, tag="res", bufs=2, name="x3")

            def ev_x3(nt, key, ps):
                nc.vector.scalar_tensor_tensor(
                    out=x3[:, nt, :], in0=ps[:], scalar=bso_c[:, nt:nt + 1],
                    in1=x2[:, nt, :], op0=OP.add, op1=OP.add)

            gemm_feat(dp["wso"], C8, [(lambda c: oT2[:, c, :], 0)], ev_x3)
            dbg_dump("s_own", lambda c: s_own[:, c, :])
            dbg_dump("s_rem", lambda c: s_rem[:, c, :])
            dbg_dump("oT2", lambda c: oT2[:, c, :])
            dbg_dump("x3", lambda c: x3[:, c, :])

        # ================= Phase 3: MLP =================
        with tc.tile_pool(name="p3", bufs=1) as p3:
            m_in = p3.tile([128, C8, T], BF16, name="m_in")
            layer_norm(lambda c: x3[:, c, :], m_in, slice(0, 512), p3)

            hT = p3.tile([128, 32, T], BF16, name="hT")

            def ev_h(ht, key, ps):
                nc.scalar.activation(out=hT[:, ht, :], in_=ps[:],
                                     func=AF.Gelu_apprx_tanh,
                                     bias=b1_c[:, ht:ht + 1], scale=1.0)

            gemm_feat(dp["w1"], 32, [(lambda c: m_in[:, c, :], 0)], ev_h)

            for nt in range(C8):
                w2t = p3.tile([128, 32, 128], BF16, tag="w2t", bufs=2,
                              name="w2t")
                nc.sync.dma_start(out=w2t[:], in_=dp["w2"][nt])
                ps = pp.tile([128, 512], F32, tag="mm", bufs=3, name="ops2")
                for kk in range(32):
                    nc.tensor.matmul(ps[:], w2t[:, kk, :], hT[:, kk, :],
                                     start=(kk == 0), stop=(kk == 31))
                ot = p3.tile([128, 512], F32R, tag="ot", bufs=2, name="ot")
                nc.vector.tensor_scalar_add(ot[:], ps[:],
                                            scalar1=b2_c[:, nt:nt + 1])
                if not KDBG:
                    nc.sync.dma_start(out=dp["outT"][:, nt, :], in_=ot[:])


def _get_program():
    if "nc" not in _PROGRAM_CACHE:
        _PROGRAM_CACHE["nc"] = _build_program()
    return _PROGRAM_CACHE["nc"]


def _tile_w(w):
    """[Din, NT*128] f32 -> [NT, 128, Din//128, 128] bf16 contiguous."""
    din, dout = w.shape
    cn, nt = din // 128, dout // 128
    t = w.reshape(cn, 128, nt, 128).transpose(2, 1, 0, 3)
    return np.ascontiguousarray(t).astype(ml_dtypes.bfloat16)


def _mov_w(w):
    """[Din, N] f32 -> [128, Din//128, N] bf16 (moving-operand layout)."""
    din, n = w.shape
    cn = din // 128
    t = w.reshape(cn, 128, n).transpose(1, 0, 2)
    return np.ascontiguousarray(t).astype(ml_dtypes.bfloat16)


def _chunk_fm(a):
    """[n_tok, D] f32 -> feature-major chunked [128, C8, n_tok]."""
    ntok = a.shape[0]
    t = a.T.reshape(C8, 128, ntok).transpose(1, 0, 2)
    return np.ascontiguousarray(t)


def kernel(**inputs) -> np.ndarray:
    from concourse.bass_utils import run_bass_kernel_spmd

    nc = _get_program()

    f32 = lambda a: np.asarray(a, np.float32)
    x = f32(inputs["x"])
    key_val = f32(inputs["key_val"])
    ln1_s, ln1_b = f32(inputs["ln1_s"]), f32(inputs["ln1_b"])
    ln2_s, ln2_b = f32(inputs["ln2_s"]), f32(inputs["ln2_b"])
    ln3_s, ln3_b = f32(inputs["ln3_s"]), f32(inputs["ln3_b"])
    ln4_s, ln4_b = f32(inputs["ln4_s"]), f32(inputs["ln4_b"])
    Wq, Wkv, Wco = f32(inputs["Wq"]), f32(inputs["Wkv"]), f32(inputs["Wco"])
    Wqkv, Wso = f32(inputs["Wqkv"]), f32(inputs["Wso"])
    W1, W2 = f32(inputs["W1"]), f32(inputs["W2"])
    bco, bso = f32(inputs["bco"]), f32(inputs["bso"])
    b1, b2 = f32(inputs["b1"]), f32(inputs["b2"])

    Wk1, Wv1 = Wkv[:, :D], Wkv[:, D:]
    Wq2, Wk2, Wv2 = Wqkv[:, :D], Wqkv[:, D:2 * D], Wqkv[:, 2 * D:]

    # LN affine folding: (x_hat * s + b) @ W = x_hat @ (diag(s) W) + b @ W.
    # K-projection bias drops (softmax shift invariance); V-projection bias
    # passes through row-normalized softmax and folds into the next bias.
    shared = {
        "wq": _tile_w(ln1_s[:, None] * Wq),
        "wk1": _tile_w(ln2_s[:, None] * Wk1),
        "wv1": _mov_w(ln2_s[:, None] * Wv1),
        "wco": _tile_w(Wco),
        "wq2": _tile_w(ln3_s[:, None] * Wq2),
        "wk2": _tile_w(ln3_s[:, None] * Wk2),
        "wv2": _mov_w(ln3_s[:, None] * Wv2),
        "wso": _tile_w(Wso),
        "w1": _tile_w(ln4_s[:, None] * W1),
        "w2": _tile_w(W2),
        "bq1": (ln1_b @ Wq)[None, :],
        "bq2": (ln3_b @ Wq2)[None, :],
        "bco": (bco + (ln2_b @ Wv1) @ Wco)[None, :],
        "bso": (bso + (ln3_b @ Wv2) @ Wso)[None, :],
        "b1": (b1 + ln4_b @ W1)[None, :],
        "b2": b2[None, :],
        "ones": np.ones((128, 128), np.float32),
    }
    in_maps = []
    for c in range(N_CORES):
        b, s = c // 2, c % 2
        m = dict(shared)
        m["xT"] = _chunk_fm(x[b, s * T:(s + 1) * T, :])
        m["kvT"] = _chunk_fm(key_val[b]).astype(ml_dtypes.bfloat16)
        in_maps.append(m)

    res = run_bass_kernel_spmd(nc, in_maps, list(range(N_CORES)))
    _PROGRAM_CACHE["last_result"] = res

    out = np.empty((B, NSEQ, D), np.float32)
    for c in range(N_CORES):
        b, s = c // 2, c % 2
        o = np.asarray(res.results[c]["outT"], np.float32)  # [128, C8, T]
        out[b, s * T:(s + 1) * T, :] = o.transpose(2, 1, 0).reshape(T, D)
    return out


# revision 16
# speedup vs baseline: 1.9246x; 1.0807x over previous
"""CrossAttnBlock kernel for 8 Trainium2 NeuronCores.

Sharding: core c -> (batch b = c//2, token-half s = c%2), 512 query tokens
per core. Cross-attention K/V is computed fully per core (duplicated within
the pair); after cross-attention the per-core residual x2 is exchanged with
a PAIR-wise AllGather (bf16) so each core rebuilds the partner half's
self-attn K/V locally.

All activations are feature-major ([feature, token]). Weights are pre-tiled
on the host into contiguous [nt, 128, C8*128] bf16 blocks so each weight
DMA is a single large contiguous transfer. LayerNorm affine params are
folded into the weights on the host (diag(s) @ W row-scaling); the LN bias
contributions become per-output-feature biases (K-projection bias drops
exactly — softmax is invariant to per-query score shifts; V-projection bias
passes through the normalized softmax unchanged and folds into the output
projection bias). K/V/x2 stay SBUF-resident. Softmax uses a ones-column in
V for the sum-exp, reciprocal_approx_fast on DVE, and a PE-matmul broadcast
of 1/z (no DRAM round trips).
"""
import sys

sys.path.insert(0, '/opt/trn_rl_repo')

import ml_dtypes
import numpy as np
import concourse.bass as bass
from concourse import bacc
import concourse.tile as tile
from concourse import mybir

F32R = mybir.dt.float32r
F32 = mybir.dt.float32
BF16 = mybir.dt.bfloat16
AF = mybir.ActivationFunctionType
OP = mybir.AluOpType

N_CORES = 8
B, NSEQ, D, H, HD = 4, 1024, 1024, 16, 64
T = 512            # tokens owned per core
TF = 1024          # full token count per batch
C8 = D // 128      # feature chunks
SCALE = 1.0 / float(np.sqrt(np.float32(HD)))
EPS = 1e-6

_PROGRAM_CACHE = {}
import os
KDBG = os.environ.get("KDBG", "")


def _build_program():
    nc = bacc.Bacc("TRN2", target_bir_lowering=False, debug=False,
                   num_devices=N_CORES)

    dp = {}
    dp["xT"] = nc.declare_dram_parameter("xT", [128, C8, T], F32R,
                                         isOutput=False)
    dp["kvT"] = nc.declare_dram_parameter("kvT", [128, C8, TF], BF16,
                                          isOutput=False)
    # pre-tiled bf16 weights: [nt, 128, C8(contraction chunks), 128]
    for nm, ntiles, nchunk in [("wq", 8, 8), ("wk1", 8, 8), ("wco", 8, 8),
                               ("wq2", 8, 8), ("wk2", 8, 8), ("wso", 8, 8),
                               ("w1", 32, 8), ("w2", 8, 32)]:
        dp[nm] = nc.declare_dram_parameter(nm, [ntiles, 128, nchunk, 128],
                                           BF16, isOutput=False)
    # V-projection weights in moving layout [128, C8, 1024]
    dp["wv1"] = nc.declare_dram_parameter("wv1", [128, C8, TF], BF16,
                                          isOutput=False)
    dp["wv2"] = nc.declare_dram_parameter("wv2", [128, C8, TF], BF16,
                                          isOutput=False)
    for nm, n in [("bqp1", H * 128), ("bqp2", H * 128), ("mask2", 256),
                  ("bco", D), ("bso", D), ("b1", 4 * D), ("b2", D)]:
        dp[nm] = nc.declare_dram_parameter(nm, [1, n], F32, isOutput=False)
    dp["ones"] = nc.declare_dram_parameter("ones", [128, 128], F32R,
                                           isOutput=False)
    dp["outT"] = nc.declare_dram_parameter("outT", [128, C8, T], F32R,
                                           isOutput=True)

    with tile.TileContext(nc) as tc:
        _emit(nc, tc, dp)
    nc.compile()
    return nc


def _emit(nc, tc, dp):
    import contextlib

    ctx = contextlib.ExitStack()
    with ctx:
        consts = ctx.enter_context(tc.tile_pool(name="consts", bufs=1))
        outer = ctx.enter_context(tc.tile_pool(name="outer", bufs=1))
        work = ctx.enter_context(tc.tile_pool(name="work", bufs=1))
        pp = ctx.enter_context(tc.tile_pool(name="pp", bufs=1, space="PSUM"))
        small = ctx.enter_context(tc.tile_pool(name="small", bufs=1))
        dramp = ctx.enter_context(tc.tile_pool(name="dramp", bufs=1,
                                               space="DRAM"))

        # ---------- constants ----------
        ones_sb = consts.tile([128, 128], F32R)
        nc.sync.dma_start(out=ones_sb[:], in_=dp["ones"][:])
        ones_col = ones_sb[:, 0:1]
        ones_row = ones_sb[0:1, :]
        ones_bf = consts.tile([128, 128], BF16)
        nc.vector.memset(ones_bf[:], 1.0)
        eps_t = consts.tile([1, 1], F32)
        nc.vector.memset(eps_t[:], EPS)

        def load_col(name, nchunk):
            col = consts.tile([128, nchunk], F32, name=f"col_{name}")
            nc.sync.dma_start(
                out=col[:], in_=dp[name].rearrange("o (c p) -> p (o c)", p=128))
            return col

        bqp1_c = load_col("bqp1", H)
        bqp2_c = load_col("bqp2", H)
        mask2_c = load_col("mask2", 2)
        bco_c = load_col("bco", C8)
        bso_c = load_col("bso", C8)
        b1_c = load_col("b1", 32)
        b2_c = load_col("b2", C8)

        pid = nc.sync.partition_id()
        partner_slot = 1 - pid % 2

        # ---------- DRAM intermediates ----------
        x2bf_d = dramp.tile([128, C8 * T], BF16, name="x2bf_d")
        ag2 = dramp.tile([2, 128, C8 * T], BF16, name="ag2")

        # ---------- generic helpers ----------
        def layer_norm(src_fn, dst, dst_sl, pool, src_bf=False):
            """Pure LN (no affine) over the feature axis for 512 tokens.

            src_fn(c) -> [128, 512] AP. dst: [128, C8, *] SBUF tile.
            src_bf: True when the source tiles are bf16 (matmul operands
            must match width, so the stats lhsT must be bf16 too).
            """
            stat1 = ones_bf[:, :] if src_bf else ones_sb[:, :]
            stats_x = pp.tile([128, 512], F32, tag="sps", bufs=2,
                              name="stats_x")
            stats_q = pp.tile([128, 512], F32, tag="ops", bufs=2,
                              name="stats_q")
            srcs = []
            for c in range(C8):
                xc = src_fn(c)
                srcs.append(xc)
                sq = work.tile([128, 512], BF16, tag="sq", bufs=3, name="sq")
                nc.scalar.activation(out=sq[:], in_=xc, func=AF.Square)
                nc.tensor.matmul(stats_x[:], stat1, xc,
                                 start=(c == 0), stop=(c == C8 - 1),
                                 skip_group_check=True)
                nc.tensor.matmul(stats_q[:], ones_bf[:, :], sq[:],
                                 start=(c == 0), stop=(c == C8 - 1),
                                 skip_group_check=True)
            mean = small.tile([1, 512], F32, tag="mean", bufs=1, name="mean")
            nc.vector.tensor_scalar_mul(mean[:], stats_x[0:1, :], 1.0 / D)
            var = small.tile([1, 512], F32, tag="var", bufs=1, name="var")
            nc.vector.tensor_scalar_mul(var[:], stats_q[0:1, :], 1.0 / D)
            m2 = small.tile([1, 512], F32, tag="m2", bufs=1, name="m2")
            nc.vector.tensor_mul(m2[:], mean[:], mean[:])
            nc.vector.tensor_sub(var[:], var[:], m2[:])
            std = small.tile([1, 512], F32, tag="std", bufs=1, name="std")
            nc.scalar.activation(out=std[:], in_=var[:], func=AF.Sqrt,
                                 bias=eps_t[:])
            inv = small.tile([1, 512], F32, tag="inv", bufs=2, name="inv")
            nc.vector.reciprocal_approx_fast(out=inv[:], in_=std[:])
            inv_bf = small.tile([1, 512], BF16, tag="inv_bf", bufs=2,
                                name="inv_bf")
            nc.scalar.copy(out=inv_bf[:], in_=inv[:])
            negminv = small.tile([1, 512], BF16, tag="negminv", bufs=2,
                                 name="negminv")
            nc.vector.scalar_tensor_tensor(
                out=negminv[:], in0=mean[:], scalar=-1.0, in1=inv[:],
                op0=OP.mult, op1=OP.mult)
            a0 = pp.tile([128, 512], F32, tag="mm", bufs=3, name="a0")
            nc.tensor.matmul(a0[:], ones_bf[0:1, :], inv_bf[:],
                             start=True, stop=True)
            c0 = pp.tile([128, 512], F32, tag="mm", bufs=3, name="c0")
            nc.tensor.matmul(c0[:], ones_bf[0:1, :], negminv[:],
                             start=True, stop=True)
            for c in range(C8):
                nc.vector.tensor_mul(dst[:, c, dst_sl], srcs[c], a0[:])
                nc.vector.tensor_add(dst[:, c, dst_sl], dst[:, c, dst_sl],
                                     c0[:])

        def gemm_feat(w_dram, n_tiles, rhs_list, evict):
            """Feature-major GEMM. w_dram: [nt, 128, C8, 128] bf16 tiles.
            rhs_list: [(rhs_fn(c) -> [128,512] AP, key)]. evict(nt, key, ps)."""
            for nt in range(n_tiles):
                wt = work.tile([128, C8, 128], BF16, tag="wt", bufs=3,
                               name="wt")
                nc.sync.dma_start(out=wt[:], in_=w_dram[nt])
                for (rhs_fn, key) in rhs_list:
                    ps = pp.tile([128, 512], F32, tag="mm", bufs=3, name="gps")
                    for c in range(C8):
                        nc.tensor.matmul(ps[:], wt[:, c, :], rhs_fn(c),
                                         start=(c == 0), stop=(c == C8 - 1))
                    evict(nt, key, ps)

        def build_v(src, jts, wv_sb, v_sb):
            """Token-major V into SBUF. src: [128, C8, 512] bf16 (LN out);
            jts: j-tile indices (token blocks of 128) relative to src."""
            for jt in jts:
                sl = slice((jt % 4) * 128, (jt % 4 + 1) * 128)
                ps0 = pp.tile([128, 512], F32, tag="mm", bufs=3, name="vps0")
                ps1 = pp.tile([128, 512], F32, tag="mm", bufs=3, name="vps1")
                for c in range(C8):
                    lhsT = src[:, c, sl]
                    nc.tensor.matmul(ps0[:], lhsT, wv_sb[:, c, 0:512],
                                     start=(c == 0), stop=(c == C8 - 1),
                                     skip_group_check=True)
                    nc.tensor.matmul(ps1[:], lhsT, wv_sb[:, c, 512:1024],
                                     start=(c == 0), stop=(c == C8 - 1),
                                     skip_group_check=True)
                nc.vector.tensor_copy(
                    out=v_sb[:, jt, 0:8, 0:64],
                    in_=ps0[:].rearrange("p (h e) -> p h e", h=8))
                nc.vector.tensor_copy(
                    out=v_sb[:, jt, 8:16, 0:64],
                    in_=ps1[:].rearrange("p (h e) -> p h e", h=8))

        def attention(qT, kT_sb, v_sb, oT, aname=""):
            for h in range(H):
                ch, off = h // 2, (h % 2) * 64
                o_ps = pp.tile([128, 512], F32, tag="ops", bufs=2,
                               name="o_ps")
                for jt in range(8):
                    s_ps = pp.tile([128, 512], F32, tag="sps", bufs=2,
                                   name="s_ps")
                    nc.tensor.matmul(
                        s_ps[:],
                        kT_sb[:, ch, jt * 128:(jt + 1) * 128],
                        qT[:, h, :], start=True, stop=True)
                    pt = work.tile([128, 512], BF16, tag="pt", bufs=3,
                                   name="pt")
                    nc.scalar.activation(out=pt[:], in_=s_ps[:], func=AF.Exp,
                                         scale=SCALE)
                    nc.tensor.matmul(o_ps[:], v_sb[:, jt, h, :], pt[:],
                                     start=(jt == 0), stop=(jt == 7),
                                     skip_group_check=True)
                if KDBG == aname + "oz0" and h == 0:
                    st = work.tile([128, 512], F32R, tag="dbgst", bufs=2,
                                   name="dbgst")
                    nc.scalar.copy(out=st[0:65, :], in_=o_ps[:])
                    nc.sync.dma_start(out=dp["outT"][0:65, 0, :],
                                      in_=st[0:65, :])
                zrow = small.tile([1, 512], F32, tag="zrow", bufs=2,
                                  name="zrow")
                nc.scalar.copy(out=zrow[:], in_=o_ps[64:65, :])
                zrec = small.tile([1, 512], F32, tag="zrec", bufs=2,
                                  name="zrec")
                nc.vector.reciprocal_approx_fast(out=zrec[:], in_=zrow[:])
                if KDBG == aname + "zr0" and h == 0:
                    st = work.tile([128, 512], F32R, tag="dbgst", bufs=2,
                                   name="dbgst")
                    nc.scalar.copy(out=st[0:1, :], in_=zrec[:])
                    nc.sync.dma_start(out=dp["outT"][0:1, 0, :],
                                      in_=st[0:1, :])
                zrec_bf = small.tile([1, 512], BF16, tag="zrec_bf", bufs=2,
                                     name="zrec_bf")
                nc.scalar.copy(out=zrec_bf[:], in_=zrec[:])
                zb = pp.tile([64, 512], F32, tag="mm", bufs=3, name="zb")
                nc.tensor.matmul(zb[:], ones_bf[0:1, 0:64], zrec_bf[:],
                                 start=True, stop=True)
                nc.scalar.copy(out=oT[off:off + 64, ch, :],
                               in_=o_ps[0:64, :])
                nc.vector.tensor_mul(oT[off:off + 64, ch, :],
                                     oT[off:off + 64, ch, :], zb[:])

        def dbg_dump(name, ap_fn, n=C8, bf=False):
            """If KDBG==name, copy chunks into outT."""
            if KDBG != name:
                return False
            for c in range(n):
                st = work.tile([128, 512], F32R, tag="dbgst", bufs=2,
                               name="dbgst")
                nc.scalar.copy(out=st[:], in_=ap_fn(c))
                nc.sync.dma_start(out=dp["outT"][:, c % C8, :], in_=st[:])
            return True

        # ---------- load x ----------
        x1 = outer.tile([128, C8, T], F32R, tag="res", bufs=2, name="x1")
        nc.sync.dma_start(out=x1[:], in_=dp["xT"][:])

        # ================= Phase 1: cross-attention =================
        with tc.tile_pool(name="p1", bufs=1) as p1:
            q_in = p1.tile([128, C8, T], BF16, name="q_in")
            layer_norm(lambda c: x1[:, c, :], q_in, slice(0, 512), p1)

            kv_in = p1.tile([128, C8, TF], BF16, name="kv_in")

            kvh0 = p1.tile([128, C8, 512], BF16, tag="kvh", bufs=1,
                           name="kvh")
            nc.sync.dma_start(out=kvh0[:], in_=dp["kvT"][:, :, 0:512])
            layer_norm(lambda c: kvh0[:, c, :], kv_in, slice(0, 512), p1, src_bf=True)

            qT1 = p1.tile([128, H, T], BF16, name="qT1")

            def ev_qT(qTp, col):
                def ev(nt, key, ps):
                    for k in (0, 1):
                        h = 2 * nt + k
                        nc.scalar.activation(
                            out=qTp[:, h, :], in_=ps[:], func=AF.Identity,
                            scale=mask2_c[:, k:k + 1], bias=col[:, h:h + 1])
                return ev

            gemm_feat(dp["wq"], C8, [(lambda c: q_in[:, c, :], 0)],
                      ev_qT(qT1, bqp1_c))

            kvh1 = p1.tile([128, C8, 512], BF16, tag="kvh", bufs=1,
                           name="kvh")
            nc.sync.dma_start(out=kvh1[:], in_=dp["kvT"][:, :, 512:1024])
            layer_norm(lambda c: kvh1[:, c, :], kv_in, slice(512, 1024), p1, src_bf=True)

            kT_sb = p1.tile([128, C8, TF], BF16, name="kT_sb")

            def ev_kT(nt, th, ps):
                nc.scalar.activation(
                    out=kT_sb[:, nt, th * 512:(th + 1) * 512], in_=ps[:],
                    func=AF.Copy)

            gemm_feat(dp["wk1"], C8,
                      [(lambda c: kv_in[:, c, 0:512], 0),
                       (lambda c: kv_in[:, c, 512:1024], 1)], ev_kT)

            v_sb = p1.tile([128, 8, H, 128], BF16, name="v_sb")
            nc.vector.memset(v_sb[:, :, :, 64], 1.0)
            wv_sb = work.tile([128, C8, TF], BF16, tag="wv", bufs=1,
                              name="wv_sb")
            nc.sync.dma_start(out=wv_sb[:], in_=dp["wv1"][:])
            build_v(kv_in[:, :, 0:512], range(0, 4), wv_sb, v_sb)
            build_v(kv_in[:, :, 512:1024], range(4, 8), wv_sb, v_sb)

            oT1 = p1.tile([128, C8, T], BF16, name="oT1")
            attention(qT1, kT_sb, v_sb, oT1, aname="a1")

            # x2 = x1 + Wco @ o + bco'
            x2 = outer.tile([128, C8, T], F32R, tag="res", bufs=2, name="x2")

            def ev_x2(nt, key, ps):
                nc.vector.scalar_tensor_tensor(
                    out=x2[:, nt, :], in0=ps[:], scalar=bco_c[:, nt:nt + 1],
                    in1=x1[:, nt, :], op0=OP.add, op1=OP.add)
                x2bf = work.tile([128, 512], BF16, tag="x2bf", bufs=2,
                                 name="x2bf")
                nc.scalar.activation(out=x2bf[:], in_=x2[:, nt, :],
                                     func=AF.Copy)
                nc.sync.dma_start(
                    out=x2bf_d[:, nt * 512:(nt + 1) * 512], in_=x2bf[:])

            gemm_feat(dp["wco"], C8, [(lambda c: oT1[:, c, :], 0)], ev_x2)
            dbg_dump("q_in", lambda c: q_in[:, c, :])
            dbg_dump("kv_in", lambda c: kv_in[:, c, 0:512])
            dbg_dump("kv_in1", lambda c: kv_in[:, c, 512:1024])
            dbg_dump("kT0", lambda c: kT_sb[:, c, 0:512])
            dbg_dump("oT1", lambda c: oT1[:, c, :])
            dbg_dump("x2", lambda c: x2[:, c, :])

        # ================= x2 exchange (pair AllGather, bf16) ============
        nc.gpsimd.collective_compute(
            "AllGather", OP.bypass,
            ins=[x2bf_d[:]],
            outs=[ag2[:]],
            replica_groups=[[0, 1], [2, 3], [4, 5], [6, 7]])

        # ================= Phase 2: self-attention =================
        with tc.tile_pool(name="p2", bufs=1) as p2:
            s_own = p2.tile([128, C8, T], BF16, name="s_own")
            layer_norm(lambda c: x2[:, c, :], s_own, slice(0, 512), p2)

            qT2 = p2.tile([128, H, T], BF16, name="qT2")

            gemm_feat(dp["wq2"], C8, [(lambda c: s_own[:, c, :], 0)],
                      ev_qT(qT2, bqp2_c))

            kT2_sb = p2.tile([128, C8, TF], BF16, name="kT2_sb")

            def ev_kT2(nt, half, ps):
                nc.scalar.activation(
                    out=kT2_sb[:, nt, half * 512:(half + 1) * 512], in_=ps[:],
                    func=AF.Copy)

            gemm_feat(dp["wk2"], C8, [(lambda c: s_own[:, c, :], 0)], ev_kT2)

            v2_sb = p2.tile([128, 8, H, 128], BF16, name="v2_sb")
            nc.vector.memset(v2_sb[:, :, :, 64], 1.0)
            wv2_sb = work.tile([128, C8, TF], BF16, tag="wv", bufs=1,
                               name="wv2_sb")
            nc.sync.dma_start(out=wv2_sb[:], in_=dp["wv2"][:])
            build_v(s_own, range(0, 4), wv2_sb, v2_sb)

            # remote half (depends on the AllGather)
            ag_sb = p2.tile([128, C8, T], BF16, name="ag_sb")
            nc.sync.dma_start(
                out=ag_sb[:],
                in_=ag2[bass.ds(partner_slot, 1), :, :].rearrange(
                    "o p (c t) -> p (o c) t", c=C8))
            s_rem = p2.tile([128, C8, T], BF16, name="s_rem")
            layer_norm(lambda c: ag_sb[:, c, :], s_rem, slice(0, 512), p2, src_bf=True)
            gemm_feat(dp["wk2"], C8, [(lambda c: s_rem[:, c, :], 1)], ev_kT2)
            build_v(s_rem, range(4, 8), wv2_sb, v2_sb)

            oT2 = p2.tile([128, C8, T], BF16, name="oT2")
            attention(qT2, kT2_sb, v2_sb, oT2, aname="a2")

            x3 = outer.tile([128, C8, T], F32R, tag="res", bufs=2, name="x3")

            def ev_x3(nt, key, ps):
                nc.vector.scalar_tensor_tensor(
                    out=x3[:, nt, :], in0=ps[:], scalar=bso_c[:, nt:nt + 1],
                    in1=x2[:, nt, :], op0=OP.add, op1=OP.add)

            gemm_feat(dp["wso"], C8, [(lambda c: oT2[:, c, :], 0)], ev_x3)
            dbg_dump("s_own", lambda c: s_own[:, c, :])
            dbg_dump("s_rem", lambda c: s_rem[:, c, :])
            dbg_dump("oT2", lambda c: oT2[:, c, :])
            dbg_dump("x3", lambda c: x3[:, c, :])

        # ================= Phase 3: MLP =================
        with tc.tile_pool(name="p3", bufs=1) as p3:
            m_in = p3.tile([128, C8, T], BF16, name="m_in")
            layer_norm(lambda c: x3[:, c, :], m_in, slice(0, 512), p3)

            hT = p3.tile([128, 32, T], BF16, name="hT")

            def ev_h(ht, key, ps):
                nc.scalar.activation(out=hT[:, ht, :], in_=ps[:],
                                     func=AF.Gelu_apprx_tanh,
                                     bias=b1_c[:, ht:ht + 1], scale=1.0)

            gemm_feat(dp["w1"], 32, [(lambda c: m_in[:, c, :], 0)], ev_h)

            for nt in range(C8):
                w2t = p3.tile([128, 32, 128], BF16, tag="w2t", bufs=2,
                              name="w2t")
                nc.sync.dma_start(out=w2t[:], in_=dp["w2"][nt])
                ps = pp.tile([128, 512], F32, tag="mm", bufs=3, name="ops2")
                for kk in range(32):
                    nc.tensor.matmul(ps[:], w2t[:, kk, :], hT[:, kk, :],
                                     start=(kk == 0), stop=(kk == 31))
                ot = p3.tile([128, 512], F32R, tag="ot", bufs=2, name="ot")
                nc.vector.tensor_scalar_add(ot[:], ps[:],
                                            scalar1=b2_c[:, nt:nt + 1])
                if not KDBG:
                    nc.sync.dma_start(out=dp["outT"][:, nt, :], in_=ot[:])


def _get_program():
    if "nc" not in _PROGRAM_CACHE:
        _PROGRAM_CACHE["nc"] = _build_program()
    return _PROGRAM_CACHE["nc"]


def _tile_w(w):
    """[Din, NT*128] f32 -> [NT, 128, Din//128, 128] bf16 contiguous."""
    din, dout = w.shape
    cn, nt = din // 128, dout // 128
    t = w.reshape(cn, 128, nt, 128).transpose(2, 1, 0, 3)
    return np.ascontiguousarray(t).astype(ml_dtypes.bfloat16)


def _mov_w(w):
    """[Din, N] f32 -> [128, Din//128, N] bf16 (moving-operand layout)."""
    din, n = w.shape
    cn = din // 128
    t = w.reshape(cn, 128, n).transpose(1, 0, 2)
    return np.ascontiguousarray(t).astype(ml_dtypes.bfloat16)


def _headpad_bias(b):
    """[D] bias -> [1, H*128]: col h holds bias on its active 64 rows."""
    out = np.zeros((128, H), np.float32)
    for h in range(H):
        ch, off = h // 2, (h % 2) * 64
        out[off:off + 64, h] = b[ch * 128 + off:ch * 128 + off + 64]
    # load_col reads [1, (c p)] with p=128 -> transpose back
    return out.T.reshape(1, H * 128)


def _halves_mask():
    m = np.zeros((128, 2), np.float32)
    m[0:64, 0] = 1.0
    m[64:128, 1] = 1.0
    return m.T.reshape(1, 256)


def _chunk_fm(a):
    """[n_tok, D] f32 -> feature-major chunked [128, C8, n_tok]."""
    ntok = a.shape[0]
    t = a.T.reshape(C8, 128, ntok).transpose(1, 0, 2)
    return np.ascontiguousarray(t)


def kernel(**inputs) -> np.ndarray:
    from concourse.bass_utils import run_bass_kernel_spmd

    nc = _get_program()

    f32 = lambda a: np.asarray(a, np.float32)
    x = f32(inputs["x"])
    key_val = f32(inputs["key_val"])
    ln1_s, ln1_b = f32(inputs["ln1_s"]), f32(inputs["ln1_b"])
    ln2_s, ln2_b = f32(inputs["ln2_s"]), f32(inputs["ln2_b"])
    ln3_s, ln3_b = f32(inputs["ln3_s"]), f32(inputs["ln3_b"])
    ln4_s, ln4_b = f32(inputs["ln4_s"]), f32(inputs["ln4_b"])
    Wq, Wkv, Wco = f32(inputs["Wq"]), f32(inputs["Wkv"]), f32(inputs["Wco"])
    Wqkv, Wso = f32(inputs["Wqkv"]), f32(inputs["Wso"])
    W1, W2 = f32(inputs["W1"]), f32(inputs["W2"])
    bco, bso = f32(inputs["bco"]), f32(inputs["bso"])
    b1, b2 = f32(inputs["b1"]), f32(inputs["b2"])

    Wk1, Wv1 = Wkv[:, :D], Wkv[:, D:]
    Wq2, Wk2, Wv2 = Wqkv[:, :D], Wqkv[:, D:2 * D], Wqkv[:, 2 * D:]

    # LN affine folding: (x_hat * s + b) @ W = x_hat @ (diag(s) W) + b @ W.
    # K-projection bias drops (softmax shift invariance); V-projection bias
    # passes through row-normalized softmax and folds into the next bias.
    shared = {
        "wq": _tile_w(ln1_s[:, None] * Wq),
        "wk1": _tile_w(ln2_s[:, None] * Wk1),
        "wv1": _mov_w(ln2_s[:, None] * Wv1),
        "wco": _tile_w(Wco),
        "wq2": _tile_w(ln3_s[:, None] * Wq2),
        "wk2": _tile_w(ln3_s[:, None] * Wk2),
        "wv2": _mov_w(ln3_s[:, None] * Wv2),
        "wso": _tile_w(Wso),
        "w1": _tile_w(ln4_s[:, None] * W1),
        "w2": _tile_w(W2),
        "bqp1": _headpad_bias(ln1_b @ Wq),
        "bqp2": _headpad_bias(ln3_b @ Wq2),
        "mask2": _halves_mask(),
        "bco": (bco + (ln2_b @ Wv1) @ Wco)[None, :],
        "bso": (bso + (ln3_b @ Wv2) @ Wso)[None, :],
        "b1": (b1 + ln4_b @ W1)[None, :],
        "b2": b2[None, :],
        "ones": np.ones((128, 128), np.float32),
    }
    in_maps = []
    for c in range(N_CORES):
        b, s = c // 2, c % 2
        m = dict(shared)
        m["xT"] = _chunk_fm(x[b, s * T:(s + 1) * T, :])
        m["kvT"] = _chunk_fm(key_val[b]).astype(ml_dtypes.bfloat16)
        in_maps.append(m)

    res = run_bass_kernel_spmd(nc, in_maps, list(range(N_CORES)))
    _PROGRAM_CACHE["last_result"] = res

    out = np.empty((B, NSEQ, D), np.float32)
    for c in range(N_CORES):
        b, s = c // 2, c % 2
        o = np.asarray(res.results[c]["outT"], np.float32)  # [128, C8, T]
        out[b, s * T:(s + 1) * T, :] = o.transpose(2, 1, 0).reshape(T, D)
    return out
